# revision 4
# baseline (speedup 1.0000x reference)
"""Trainium2 Bass kernel for Bahdanau-style additive attention.

    h_proj = hidden @ W_attn[:H] + b_attn                # (B, H)
    e_proj = encoder_outputs @ W_attn[H:]                # (B, S, H)
    energy = tanh(h_proj[:, None, :] + e_proj)           # (B, S, H)
    att    = energy @ v                                  # (B, S)
    out    = softmax(att, axis=1)                        # (B, S)

B=32, S=2048, H=1024. Data-parallel over batch: 4 batches per core on 8
NeuronCores. Per-core kernel (all matmul inputs bf16, fp32 accumulation):

  - encoder rows stream in via SWDGE cast-DMA (fp32->bf16), then an xbar
    SBUF->SBUF DMA transpose puts H on partitions: xt[p, r, k, j] =
    enc[r*128+j, k*128+p], so the PE can contract over H.
  - e_proj^T tiles [h_out=128, s=512] accumulate over 8 k-tiles in PSUM,
    with We tiles stationary (native [h_in, h_out] layout, no transpose).
  - ScalarE fuses the h_proj bias add + tanh in one pass (bias is
    per-partition in this layout), writing bf16 to SBUF.
  - The v-dot is a second matmul (lhsT = v^T column, M=1) accumulating
    logits [1, 512] over the 8 h-tiles in PSUM.
  - Softmax (exp, sum, reciprocal, scale) runs on one partition per batch.
"""
import numpy as np

B, S, H = 32, 2048, 1024
N_CORES = 8
B_LOCAL = B // N_CORES          # 4 batches per core
SL = B_LOCAL * S                # 8192 encoder rows per core
KT = H // 128                   # 8 contraction tiles
MT = H // 128                   # 8 output-H tiles
S_CHUNK = 512
RT = S_CHUNK // 128             # 4 row sub-tiles per chunk
N_CHUNKS = S // S_CHUNK         # 4 chunks per batch

_CACHE = {}


def _build():
    import concourse.mybir as mybir
    import concourse.tile as tile
    from concourse import bacc

    f32 = mybir.dt.float32
    bf16 = mybir.dt.bfloat16

    nc = bacc.Bacc("TRN2", target_bir_lowering=False, debug=False,
                   num_devices=N_CORES)
    enc = nc.dram_tensor("enc", [SL, H], f32, kind="ExternalInput").ap()
    hidden = nc.dram_tensor("hidden", [B_LOCAL, H], f32, kind="ExternalInput").ap()
    w_attn = nc.dram_tensor("w_attn", [2 * H, H], f32, kind="ExternalInput").ap()
    b_attn = nc.dram_tensor("b_attn", [H], f32, kind="ExternalInput").ap()
    v_in = nc.dram_tensor("v", [H], f32, kind="ExternalInput").ap()
    out = nc.dram_tensor("out", [B_LOCAL, S], f32, kind="ExternalOutput").ap()

    with tile.TileContext(nc) as tc:
        with (
            tc.tile_pool(name="weights", bufs=1) as w_pool,
            tc.tile_pool(name="small", bufs=1) as small_pool,
            tc.tile_pool(name="dram", bufs=1, space="DRAM") as dram_pool,
            tc.tile_pool(name="raw", bufs=2) as raw_pool,
            tc.tile_pool(name="xt", bufs=2) as xt_pool,
            tc.tile_pool(name="tanh", bufs=4) as tanh_pool,
            tc.tile_pool(name="perbatch", bufs=2) as pb_pool,
            tc.tile_pool(name="psum_e", bufs=4, space="PSUM") as psum_e_pool,
            tc.tile_pool(name="psum_l", bufs=3, space="PSUM") as psum_l_pool,
        ):
            # ---- weights (cast to bf16 during DMA) ----
            wh_sb = w_pool.tile([128, KT, H], bf16)   # W_attn[:H]  [h_in, h_out]
            we_sb = w_pool.tile([128, KT, H], bf16)   # W_attn[H:]  [h_in, h_out]
            nc.gpsimd.dma_start(
                out=wh_sb[:], in_=w_attn[:H].rearrange("(k p) h -> p k h", p=128))
            nc.gpsimd.dma_start(
                out=we_sb[:], in_=w_attn[H:].rearrange("(k p) h -> p k h", p=128))

            # ---- small constants ----
            # b_attn^T : [128, KT] fp32 (per-partition bias columns)
            bt_sb = small_pool.tile([128, KT], f32)
            nc.gpsimd.dma_start(out=bt_sb[:],
                                in_=b_attn.rearrange("(t p) -> p t", p=128))
            # v^T : [128, KT] bf16 (stationary columns for the v-dot)
            vt_sb = small_pool.tile([128, KT], bf16)
            nc.gpsimd.dma_start(out=vt_sb[:],
                                in_=v_in.rearrange("(t p) -> p t", p=128))

            # ---- hidden^T via rearranged-AP cast DMA (tiny) ----
            ht_sb = small_pool.tile([128, KT, B_LOCAL], bf16)   # [h_in, k, b]
            for k in range(KT):
                nc.gpsimd.dma_start(
                    out=ht_sb[:, k, :],
                    in_=hidden[:, k * 128:(k + 1) * 128].rearrange("b p -> p b"))

            # ---- h_proj^T = (hidden @ Wh + b)^T : [h_out, b] per m-tile ----
            hp_sb = small_pool.tile([128, MT, B_LOCAL], f32)
            for m in range(MT):
                ps_hp = psum_l_pool.tile([128, B_LOCAL], f32, tag="psl")
                for k in range(KT):
                    nc.tensor.matmul(ps_hp[:],
                                     wh_sb[:, k, m * 128:(m + 1) * 128],
                                     ht_sb[:, k, :],
                                     start=(k == 0), stop=(k == KT - 1))
                nc.vector.tensor_scalar_add(out=hp_sb[:, m, :], in0=ps_hp[:],
                                            scalar1=bt_sb[:, m:m + 1])

            # ---- main loop over (batch, s-chunk) ----
            for b in range(B_LOCAL):
                bg = b + 0  # global batch index within this core's shard
                ex_sb = pb_pool.tile([1, S], f32, tag="ex")
                for c in range(N_CHUNKS):
                    base = b * S + c * S_CHUNK
                    # load 512 encoder rows, cast fp32->bf16
                    raw = raw_pool.tile([128, RT, H], bf16)
                    nc.gpsimd.dma_start(
                        out=raw[:],
                        in_=enc[base:base + S_CHUNK, :].rearrange(
                            "(r p) h -> p r h", p=128))
                    # xbar transpose: xt[p, r, k, j] = raw[j, r, k*128+p]
                    xt = xt_pool.tile([128, RT, KT, 128], bf16)
                    for r in range(RT):
                        nc.sync.dma_start_transpose(xt[:, r], raw[:, r, :])

                    psl = psum_l_pool.tile([1, S_CHUNK], f32, tag="psl")
                    for m in range(MT):
                        pse = psum_e_pool.tile([128, S_CHUNK], f32)
                        for k in range(KT):
                            nc.tensor.matmul(pse[:],
                                             we_sb[:, k, m * 128:(m + 1) * 128],
                                             xt[:, :, k, :],
                                             start=(k == 0), stop=(k == KT - 1))
                        th = tanh_pool.tile([128, S_CHUNK], bf16)
                        nc.scalar.activation(
                            out=th[:], in_=pse[:],
                            func=mybir.ActivationFunctionType.Tanh,
                            bias=hp_sb[:, m, bg:bg + 1], scale=1.0)
                        nc.tensor.matmul(psl[:], vt_sb[:, m:m + 1], th[:],
                                         start=(m == 0), stop=(m == MT - 1))
                    # exp of logits chunk straight out of PSUM
                    nc.scalar.activation(
                        out=ex_sb[0:1, c * S_CHUNK:(c + 1) * S_CHUNK],
                        in_=psl[:], func=mybir.ActivationFunctionType.Exp)
                # softmax normalize (no max-subtraction needed: |logit| <= ~26)
                sm = pb_pool.tile([1, 2], f32, tag="sm")
                nc.vector.reduce_sum(out=sm[0:1, 0:1], in_=ex_sb[:],
                                     axis=mybir.AxisListType.X)
                nc.vector.reciprocal(out=sm[0:1, 1:2], in_=sm[0:1, 0:1])
                ot = pb_pool.tile([1, S], f32, tag="ot")
                nc.vector.tensor_scalar_mul(out=ot[:], in0=ex_sb[:],
                                            scalar1=sm[0:1, 1:2])
                nc.sync.dma_start(out=out[b:b + 1, :], in_=ot[:])

    nc.compile()
    return nc


def _build_runner():
    """Compile once and build a persistent jitted SPMD executor."""
    import jax
    from jax.sharding import Mesh, PartitionSpec
    from jax.experimental.shard_map import shard_map
    import concourse.mybir as mybir
    from concourse import bass2jax

    nc = _build()
    bass2jax.install_neuronx_cc_hook()

    partition_name = nc.partition_id_tensor.name if nc.partition_id_tensor else None
    in_names, out_names, out_avals, zero_outs = [], [], [], []
    for alloc in nc.m.functions[0].allocations:
        if not isinstance(alloc, mybir.MemoryLocationSet):
            continue
        name = alloc.memorylocations[0].name
        if alloc.kind == "ExternalInput":
            if name != partition_name:
                in_names.append(name)
        elif alloc.kind == "ExternalOutput":
            out_names.append(name)
            shape = tuple(alloc.tensor_shape)
            dtype = mybir.dt.np(alloc.dtype)
            out_avals.append(jax.core.ShapedArray(shape, dtype))
            zero_outs.append(np.zeros(shape, dtype))
    n_params = len(in_names)
    n_outs = len(out_avals)
    in_names = list(in_names) + list(out_names)
    if partition_name is not None:
        in_names.append(partition_name)
    donate = tuple(range(n_params, n_params + n_outs))

    def _body(*args):
        operands = list(args)
        if partition_name is not None:
            operands.append(bass2jax.partition_id_tensor())
        outs = bass2jax._bass_exec_p.bind(
            *operands,
            out_avals=tuple(out_avals),
            in_names=tuple(in_names),
            out_names=tuple(out_names),
            lowering_input_output_aliases=(),
            sim_require_finite=True,
            sim_require_nnan=True,
            nc=nc,
        )
        return tuple(outs)

    devices = jax.devices()[:N_CORES]
    assert len(devices) >= N_CORES, f"need {N_CORES} devices"
    mesh = Mesh(np.asarray(devices[:N_CORES]), ("core",))
    in_specs = (PartitionSpec("core"),) * (n_params + n_outs)
    out_specs = (PartitionSpec("core"),) * len(out_names)
    sharded = jax.jit(
        shard_map(_body, mesh=mesh, in_specs=in_specs, out_specs=out_specs,
                  check_rep=False),
        donate_argnums=donate, keep_unused=True)
    sharding = jax.sharding.NamedSharding(mesh, PartitionSpec("core"))

    state = {
        "sharded": sharded,
        "sharding": sharding,
        "in_names": in_names[:n_params],
        "out_names": out_names,
        "out_avals": out_avals,
        "zero_outs": zero_outs,
        "jax": jax,
    }
    return state


def _get_state():
    if "state" not in _CACHE:
        _CACHE["state"] = _build_runner()
    return _CACHE["state"]


def prepare_in_maps(hidden, encoder_outputs, W_attn, b_attn, v):
    """Shard inputs: batch-split encoder_outputs, replicate the rest."""
    enc = np.ascontiguousarray(np.asarray(encoder_outputs, dtype=np.float32))
    hid = np.ascontiguousarray(np.asarray(hidden, dtype=np.float32))
    W = np.ascontiguousarray(np.asarray(W_attn, dtype=np.float32))
    bb = np.ascontiguousarray(np.asarray(b_attn, dtype=np.float32))
    vv = np.ascontiguousarray(np.asarray(v, dtype=np.float32))
    in_maps = []
    for c in range(N_CORES):
        shard = enc[c * B_LOCAL:(c + 1) * B_LOCAL].reshape(SL, H)
        hshard = hid[c * B_LOCAL:(c + 1) * B_LOCAL]
        in_maps.append({"enc": shard, "hidden": hshard, "w_attn": W,
                        "b_attn": bb, "v": vv})
    return in_maps


def device_inputs(in_maps):
    st = _get_state()
    jax = st["jax"]
    concat_in = [
        np.concatenate([np.asarray(m[name]) for m in in_maps], axis=0)
        for name in st["in_names"]
    ]
    dev = [jax.device_put(a, st["sharding"]) for a in concat_in]
    jax.block_until_ready(dev)
    return dev


def run_device(dev_in):
    """One SPMD execution; returns the (B, S) fp32 output."""
    st = _get_state()
    jax = st["jax"]
    zeros = [
        jax.device_put(np.zeros((N_CORES * z.shape[0], *z.shape[1:]), z.dtype),
                       st["sharding"])
        for z in st["zero_outs"]
    ]
    out_arrs = st["sharded"](*dev_in, *zeros)
    jax.block_until_ready(out_arrs)
    i = st["out_names"].index("out")
    full = np.asarray(out_arrs[i]).reshape(N_CORES, B_LOCAL, S)
    return full.reshape(B, S)


def kernel(hidden, encoder_outputs, W_attn, b_attn, v):
    in_maps = prepare_in_maps(hidden, encoder_outputs, W_attn, b_attn, v)
    dev_in = device_inputs(in_maps)
    return run_device(dev_in).astype(np.float32)
